# revision 29
# baseline (speedup 1.0000x reference)
"""Trainium2 Bass kernel for nn_CRLoss (masked cosine-similarity contrastive loss).

Strategy (data-parallel over batch, 2 batches per core on 8 cores):
  Host: normalize rows in fp32, permute each batch's rows so label==0 ("fake")
  rows come first, cast to bf16, ship as [128, T] per batch.
  Device (per batch, 16 row-tiles of 128 rows):
    - fake row-tiles (rt < t_lo): 4 matmuls -> S[rows, 0:2048) in two
      [128,1024] PSUM halves; DVE tensor_reduce min over the fake-certain zone
      [0:CF) straight from PSUM (v0 partial); ACT casts the [CR:T) slice to
      bf16 and DMA ships it (it feeds the fake rows' own max-over-real stat
      AND, transposed on the host, the real rows' max-over-fake stat T0).
    - real row-tiles (rt >= t_hi): only 2 matmuls -> S[rows, 1024:2048).
      Most reduce min over [CR:T) on DVE straight from PSUM (v2 partial) and
      ship NOTHING; a few ship the [CR:T) slice instead to balance DVE vs ACT.
      The fake-zone max for real rows comes entirely from symmetry
      (S[j,c] = S[c,j]) via the fake tiles' shipped slices + straddle rows.
    - straddle row-tiles (mixed labels; they own ALL mixed-strip columns
      [CF:CR) as rows): 4 matmuls, full [0:T) row cast + DMA. These rows
      are the symmetric source for every tile's strip-column contributions.
  PSUM has exactly two consumers on trn2 (DVE one port, ACT; GPSIMD/DMA cannot
  read it), so the drain is split DVE/ACT and bulk data leaves as bf16.
  Host: min/max the shipped slices/rows in numpy (symmetric gathers), combine
  with device stats, then the reference's relu/mean/sum tail over B.
  bf16 shipping contributes ~2e-4 rel err (gate is 2e-2).
"""
import os
import sys

sys.path.insert(0, "/opt/trn_rl_repo")

import numpy as np
import ml_dtypes

B, T, D = 16, 2048, 128
NCORES = 8
BPC = B // NCORES  # batches per core
TH_SIM_MIN = 0.9
TH_DIFF_MAX = 0.1
NT128 = T // 128
HC = T // 2  # 1024: boundary between the lo and hi PSUM halves


def _ship_real(rt, s, t_hi):
    """Real tiles that ship their [CR:T) slice instead of reducing on DVE.

    Alternating by row-tile keeps per-pair consumer load smooth: a fake tile
    (DVE reduce + ACT cast) paired with a DVE-real tile overloads DVE past
    the PE's pace, stalling the PE on PSUM reuse; alternating ship/DVE reals
    lets each engine catch up every other pair."""
    return (rt - t_hi) % 2 == 1


def _build(CF, CR, t_lo, t_hi):
    import concourse.bacc as bacc
    import concourse.mybir as mybir
    import concourse.tile as tile

    f32 = mybir.dt.float32
    bf16 = mybir.dt.bfloat16
    Alu = mybir.AluOpType
    X = mybir.AxisListType.X
    NSTR = t_hi - t_lo   # straddle tiles per slot
    CRH = CR - HC        # [CR:T) start within the hi half
    WS = T - CR          # shipped slice width

    nc = bacc.Bacc("TRN2", target_bir_lowering=False)
    embt = nc.dram_tensor("embt", [BPC, 128, T], bf16, kind="ExternalInput")
    stats_c = nc.dram_tensor("stats_c", [BPC, 128, 2, NT128], f32, kind="ExternalOutput")
    shipd = nc.dram_tensor("shipd", [BPC, NT128, 128, WS], bf16, kind="ExternalOutput")
    shipf = nc.dram_tensor("shipf", [BPC, NSTR, 128, T], bf16, kind="ExternalOutput")

    with tile.TileContext(nc) as tc:
        with (
            tc.tile_pool(name="cst", bufs=1) as cst,
            tc.tile_pool(name="scr", bufs=3) as scrp,
            tc.tile_pool(name="stp", bufs=2) as stp,
            tc.tile_pool(name="ps", bufs=4, space="PSUM") as ps,
        ):
            nts = []
            for s in range(BPC):
                nt = cst.tile([128, T], bf16, tag=f"nt{s}", name=f"nt{s}")
                nts.append(nt)
            # chunk order: both slots' first chunks first, so the leading
            # matmuls of either slot unblock as early as possible (all on the
            # sync queue — gpsimd-issued DMAs measured slower end to end)
            for lo, hi in ((0, 512), (512, HC), (HC, T)):
                for s in range(BPC):
                    nc.sync.dma_start(nts[s][:, lo:hi], embt[s][:, lo:hi])

            stcs = []
            for s in range(BPC):
                stc = stp.tile([128, 2, NT128], f32, tag="stc", name=f"stc{s}")
                nc.gpsimd.memset(stc[:], 0.0)
                stcs.append(stc)

            # warmup matmuls on local garbage while the input DMAs are in
            # flight: the PE pstate needs ~3us of continuous busy to reach
            # 2.4GHz, so ramp it during the otherwise dead head
            warm = cst.tile([128, 512], bf16, tag="warm", name="warm")
            nc.gpsimd.memset(warm[:], 0.5)
            for i in range(8):
                pw = ps.tile([128, 512], f32, tag="ph", name=f"pwarm{i}")
                nc.tensor.matmul(pw[:], warm[:, 0:128], warm[:])

            # Tile order: alternate fake (4 matmuls) and real (2 matmuls)
            # tiles so PE production and consumer work stay balanced per pair;
            # straddle tiles (heavy ACT + big DMA) are spaced out.
            fakes = list(range(0, t_lo))
            reals = list(range(t_hi, NT128))
            strads = list(range(t_lo, t_hi))
            # lead with two fake tiles: they only need the first input chunks
            # (a real tile's hi half would stall on the last chunk to arrive)
            head, fk = fakes[:2], fakes[2:]
            order = list(head)
            for i in range(max(len(fk), len(reals))):
                if i < len(fk):
                    order.append(fk[i])
                if i < len(reals):
                    order.append(reals[i])
            for k, st in enumerate(strads):
                order.insert(
                    (k + 1) * (len(order) + len(strads)) // (len(strads) + 1), st
                )
            # end on reduce-only tiles (no cast, no ship DMA) so the post-PE
            # drain is short: pick real tiles that stay on DVE for both slots
            tail = [rt for rt in reals if all(
                not _ship_real(rt, s, t_hi) for s in range(BPC))][-2:]
            order = [rt for rt in order if rt not in tail] + tail

            for rt in order:
                for s in range(BPC):
                    nt = nts[s]
                    fake_t = rt < t_lo
                    strad = t_lo <= rt < t_hi
                    lhsT = nt[:, rt * 128 : (rt + 1) * 128]
                    pLo = None
                    if fake_t or strad:
                        pLo = ps.tile([128, HC], f32, tag="ph", name=f"pLo{s}_{rt}")
                        for j in range(2):
                            nc.tensor.matmul(
                                pLo[:, 512 * j : 512 * (j + 1)],
                                lhsT,
                                nt[:, 512 * j : 512 * (j + 1)],
                            )
                    pHi = ps.tile([128, HC], f32, tag="ph", name=f"pHi{s}_{rt}")
                    for j in range(2):
                        nc.tensor.matmul(
                            pHi[:, 512 * j : 512 * (j + 1)],
                            lhsT,
                            nt[:, HC + 512 * j : HC + 512 * (j + 1)],
                        )
                    if strad:
                        # full row to host; symmetric strip source for all tiles
                        shf = scrp.tile([128, T], bf16, tag="shf", name=f"shf{s}_{rt}")
                        nc.scalar.copy(shf[:, 0:HC], pLo[:])
                        nc.scalar.copy(shf[:, HC:T], pHi[:])
                        nc.sync.dma_start(shipf[s][rt - t_lo], shf[:])
                        continue
                    if fake_t:
                        # v0 partial: min over the fake-certain zone, from PSUM
                        nc.vector.tensor_reduce(
                            stcs[s][:, 0, rt : rt + 1], pLo[:, 0:CF], axis=X, op=Alu.min
                        )
                    if fake_t or _ship_real(rt, s, t_hi):
                        shp = scrp.tile([128, 1024], bf16, tag="shp", name=f"shp{s}_{rt}")
                        nc.scalar.copy(shp[:, 0:WS], pHi[:, CRH:HC])
                        nc.sync.dma_start(shipd[s][rt], shp[:, 0:WS])
                    else:
                        # v2 partial: min over [CR:T) straight from PSUM
                        nc.vector.tensor_reduce(
                            stcs[s][:, 1, rt : rt + 1], pHi[:, CRH:HC], axis=X, op=Alu.min
                        )

            for s in range(BPC):
                nc.sync.dma_start(stats_c[s], stcs[s][:])

    nc.compile()
    return nc


def _prep(embeddings, label):
    """Host preprocessing: permutations, zone bounds, bf16 packed layout."""
    perms = np.empty((B, T), dtype=np.int64)
    nfs = np.empty(B, dtype=np.int64)
    for b in range(B):
        lb = label[b]
        perms[b] = np.argsort(lb, kind="stable")
        nfs[b] = int((lb == 0).sum())
    valid = (nfs > 0) & (nfs < T)
    if not valid.any():
        return None

    CF = int(nfs[valid].min())
    CR = int(nfs[valid].max())
    # the device reduces [0:CF) from the lo half and [CR:T) from the hi half,
    # so clamp the strip to bracket the 1024 boundary
    CF = max(2, min(CF, HC))
    CR = min(T - 2, max(CR, HC))
    t_lo = CF // 128
    t_hi = (CR + 127) // 128

    w = np.sqrt(np.sum(embeddings * embeddings, axis=-1, keepdims=True))
    n = embeddings / np.maximum(w, 1e-8)

    in_maps = []
    for c in range(NCORES):
        embt = np.empty((BPC, 128, T), dtype=ml_dtypes.bfloat16)
        for s in range(BPC):
            b = c * BPC + s
            embt[s] = n[b][perms[b]].T.astype(ml_dtypes.bfloat16)
        in_maps.append({"embt": embt})
    return perms, nfs, valid, CF, CR, t_lo, t_hi, in_maps


def kernel(embeddings, label):
    embeddings = np.ascontiguousarray(np.asarray(embeddings, dtype=np.float32))
    label = np.asarray(label)
    assert embeddings.shape == (B, T, D) and label.shape == (B, T)

    prep = _prep(embeddings, label)
    if prep is None:
        return np.float32(0.0)
    perms, nfs, valid, CF, CR, t_lo, t_hi, in_maps = prep

    nc = _build(CF, CR, t_lo, t_hi)

    from concourse.bass_utils import run_bass_kernel_spmd

    trace = bool(os.environ.get("CRL_TRACE"))
    if trace:
        _install_ntff_shim()
    res = run_bass_kernel_spmd(
        nc, in_maps, core_ids=list(range(NCORES)), trace=trace
    )
    if trace and res.exec_time_ns is not None:
        print(f"HW exec time: {res.exec_time_ns} ns")
        if res.instructions_and_trace:
            print("trace:", res.instructions_and_trace[1])

    # host tail: combine device stats, shipped [CR:T) slices, straddle rows
    base = t_lo * 128
    total = 0.0
    for c in range(NCORES):
        out = res.results[c]
        for s in range(BPC):
            b = c * BPC + s
            if not valid[b]:
                continue
            nf = int(nfs[b])
            stc = out["stats_c"][s].astype(np.float64)  # [128, 2, NT]
            Sstrip = out["shipf"][s].astype(np.float64).reshape(-1, T)
            shp = out["shipd"][s].astype(np.float64)    # [NT, 128, WS] cols CR:T

            minfake = np.full(T, np.inf)
            maxreal = np.full(T, -np.inf)
            minreal = np.full(T, np.inf)
            maxfake = np.full(T, -np.inf)

            # symmetric one-shot vectors over straddle rows (fake rows
            # [base:nf) for v0/v3 strip parts, real rows [nf:..) for v1/v2)
            FS = Sstrip[0 : nf - base]
            RS = Sstrip[nf - base :]
            M0 = FS.min(0) if len(FS) else np.full(T, np.inf)
            M1 = FS.max(0) if len(FS) else np.full(T, -np.inf)
            M2 = RS.max(0) if len(RS) else np.full(T, -np.inf)
            M3 = RS.min(0) if len(RS) else np.full(T, np.inf)
            # T0[j-CR]: max over pure-fake rows of col j (v3 source, j >= CR)
            if t_lo > 0:
                T0 = shp[0:t_lo].reshape(-1, T - CR).max(0)
            else:
                T0 = np.full(T - CR, -np.inf)

            for rt in range(NT128):
                rows = slice(rt * 128, (rt + 1) * 128)
                if t_lo <= rt < t_hi:  # straddle rows: direct from full rows
                    raw = Sstrip[(rt - t_lo) * 128 : (rt - t_lo + 1) * 128]
                    minfake[rows] = raw[:, 0:nf].min(-1)
                    maxfake[rows] = raw[:, 0:nf].max(-1)
                    minreal[rows] = raw[:, nf:T].min(-1)
                    maxreal[rows] = raw[:, nf:T].max(-1)
                    continue
                if rt < t_lo:  # pure fake rows: v0, v1
                    minfake[rows] = np.minimum(stc[:, 0, rt], M0[rows])
                    maxreal[rows] = np.maximum(shp[rt].max(-1), M2[rows])
                else:  # pure real rows: v2, v3
                    if _ship_real(rt, s, t_hi):
                        v2p = shp[rt].min(-1)
                    else:
                        v2p = stc[:, 1, rt]
                    minreal[rows] = np.minimum(v2p, M3[rows])
                    maxfake[rows] = np.maximum(
                        T0[rows.start - CR : rows.stop - CR], M1[rows]
                    )
            f2f = np.maximum(TH_SIM_MIN - minfake[:nf], 0.0).mean()
            r2r = np.maximum(TH_SIM_MIN - minreal[nf:], 0.0).mean()
            f2r = np.maximum(maxreal[:nf] - TH_DIFF_MAX, 0.0).mean()
            r2f = np.maximum(maxfake[nf:] - TH_DIFF_MAX, 0.0).mean()
            total += f2f + r2r + f2r + r2f
    return np.float32(total / B)


def _install_ntff_shim():
    """antenv.axon_hooks is missing on this image; inject it so trace=True works."""
    import types

    import antenv

    if hasattr(antenv, "axon_hooks"):
        return
    from trn_agent_boot.trn_boot import _ntff_profile_via_ctypes

    mod = types.ModuleType("antenv.axon_hooks")
    mod._hook = _ntff_profile_via_ctypes("/opt/axon/libaxon_pjrt.so")
    mod.get_axon_ntff_profile_hook = lambda: mod._hook
    mod.set_axon_ntff_profile_hook = lambda h: setattr(mod, "_hook", h)
    sys.modules["antenv.axon_hooks"] = mod
    antenv.axon_hooks = mod


# revision 30
# speedup vs baseline: 1.0265x; 1.0265x over previous
"""Trainium2 Bass kernel for nn_CRLoss (masked cosine-similarity contrastive loss).

Strategy (data-parallel over batch, 2 batches per core on 8 cores):
  Host: normalize rows in fp32, permute each batch's rows so label==0 ("fake")
  rows come first, cast to bf16, ship as [128, T] per batch.
  Device (per batch, 16 row-tiles of 128 rows):
    - fake row-tiles (rt < t_lo): 4 matmuls -> S[rows, 0:2048) in two
      [128,1024] PSUM halves; DVE tensor_reduce min over the fake-certain zone
      [0:CF) straight from PSUM (v0 partial); ACT casts the [CR:T) slice to
      bf16 and DMA ships it (it feeds the fake rows' own max-over-real stat
      AND, transposed on the host, the real rows' max-over-fake stat T0).
    - real row-tiles (rt >= t_hi): only 2 matmuls -> S[rows, 1024:2048).
      Most reduce min over [CR:T) on DVE straight from PSUM (v2 partial) and
      ship NOTHING; a few ship the [CR:T) slice instead to balance DVE vs ACT.
      The fake-zone max for real rows comes entirely from symmetry
      (S[j,c] = S[c,j]) via the fake tiles' shipped slices + straddle rows.
    - straddle row-tiles (mixed labels; they own ALL mixed-strip columns
      [CF:CR) as rows): 4 matmuls, full [0:T) row cast + DMA. These rows
      are the symmetric source for every tile's strip-column contributions.
  PSUM has exactly two consumers on trn2 (DVE one port, ACT; GPSIMD/DMA cannot
  read it), so the drain is split DVE/ACT and bulk data leaves as bf16.
  Host: min/max the shipped slices/rows in numpy (symmetric gathers), combine
  with device stats, then the reference's relu/mean/sum tail over B.
  bf16 shipping contributes ~2e-4 rel err (gate is 2e-2).
"""
import os
import sys

sys.path.insert(0, "/opt/trn_rl_repo")

import numpy as np
import ml_dtypes

B, T, D = 16, 2048, 128
NCORES = 8
BPC = B // NCORES  # batches per core
TH_SIM_MIN = 0.9
TH_DIFF_MAX = 0.1
NT128 = T // 128
HC = T // 2  # 1024: boundary between the lo and hi PSUM halves


def _ship_real(rt, s, t_hi):
    """Real tiles that ship their [CR:T) slice instead of reducing on DVE."""
    return ((rt - t_hi) * BPC + s) % 4 == 3


def _build(CF, CR, t_lo, t_hi):
    import concourse.bacc as bacc
    import concourse.mybir as mybir
    import concourse.tile as tile

    f32 = mybir.dt.float32
    bf16 = mybir.dt.bfloat16
    Alu = mybir.AluOpType
    X = mybir.AxisListType.X
    NSTR = t_hi - t_lo   # straddle tiles per slot
    CRH = CR - HC        # [CR:T) start within the hi half
    WS = T - CR          # shipped slice width

    nc = bacc.Bacc("TRN2", target_bir_lowering=False)
    embt = nc.dram_tensor("embt", [BPC, 128, T], bf16, kind="ExternalInput")
    stats_c = nc.dram_tensor("stats_c", [BPC, 128, 2, NT128], f32, kind="ExternalOutput")
    shipd = nc.dram_tensor("shipd", [BPC, NT128, 128, WS], bf16, kind="ExternalOutput")
    shipf = nc.dram_tensor("shipf", [BPC, NSTR, 128, T], bf16, kind="ExternalOutput")

    with tile.TileContext(nc) as tc:
        with (
            tc.tile_pool(name="cst", bufs=1) as cst,
            tc.tile_pool(name="scr", bufs=3) as scrp,
            tc.tile_pool(name="stp", bufs=2) as stp,
            tc.tile_pool(name="ps", bufs=4, space="PSUM") as ps,
        ):
            nts = []
            for s in range(BPC):
                nt = cst.tile([128, T], bf16, tag=f"nt{s}", name=f"nt{s}")
                nts.append(nt)
            # chunk order: both slots' first chunks first, so the leading
            # matmuls of either slot unblock as early as possible (all on the
            # sync queue — gpsimd-issued DMAs measured slower end to end)
            for lo, hi in ((0, 512), (512, HC), (HC, T)):
                for s in range(BPC):
                    nc.sync.dma_start(nts[s][:, lo:hi], embt[s][:, lo:hi])

            stcs = []
            for s in range(BPC):
                stc = stp.tile([128, 2, NT128], f32, tag="stc", name=f"stc{s}")
                nc.gpsimd.memset(stc[:], 0.0)
                stcs.append(stc)

            # warmup matmuls on local garbage while the input DMAs are in
            # flight: the PE pstate needs ~3us of continuous busy to reach
            # 2.4GHz, so ramp it during the otherwise dead head
            warm = cst.tile([128, 512], bf16, tag="warm", name="warm")
            nc.gpsimd.memset(warm[:], 0.5)
            for i in range(8):
                pw = ps.tile([128, 512], f32, tag="ph", name=f"pwarm{i}")
                nc.tensor.matmul(pw[:], warm[:, 0:128], warm[:])

            # Tile order: alternate fake (4 matmuls) and real (2 matmuls)
            # tiles so PE production and consumer work stay balanced per pair;
            # straddle tiles (heavy ACT + big DMA) are spaced out.
            fakes = list(range(0, t_lo))
            reals = list(range(t_hi, NT128))
            strads = list(range(t_lo, t_hi))
            # lead with two fake tiles: they only need the first input chunks
            # (a real tile's hi half would stall on the last chunk to arrive)
            head, fk = fakes[:2], fakes[2:]
            order = list(head)
            for i in range(max(len(fk), len(reals))):
                if i < len(fk):
                    order.append(fk[i])
                if i < len(reals):
                    order.append(reals[i])
            for k, st in enumerate(strads):
                order.insert(
                    (k + 1) * (len(order) + len(strads)) // (len(strads) + 1), st
                )
            # end on reduce-only tiles (no cast, no ship DMA) so the post-PE
            # drain is short: pick real tiles that stay on DVE for both slots
            tail = [rt for rt in reals if all(
                not _ship_real(rt, s, t_hi) for s in range(BPC))][-2:]
            order = [rt for rt in order if rt not in tail] + tail

            for rt in order:
                for s in range(BPC):
                    nt = nts[s]
                    fake_t = rt < t_lo
                    strad = t_lo <= rt < t_hi
                    lhsT = nt[:, rt * 128 : (rt + 1) * 128]
                    pLo = None
                    if fake_t or strad:
                        pLo = ps.tile([128, HC], f32, tag="ph", name=f"pLo{s}_{rt}")
                        for j in range(2):
                            nc.tensor.matmul(
                                pLo[:, 512 * j : 512 * (j + 1)],
                                lhsT,
                                nt[:, 512 * j : 512 * (j + 1)],
                            )
                    pHi = ps.tile([128, HC], f32, tag="ph", name=f"pHi{s}_{rt}")
                    for j in range(2):
                        nc.tensor.matmul(
                            pHi[:, 512 * j : 512 * (j + 1)],
                            lhsT,
                            nt[:, HC + 512 * j : HC + 512 * (j + 1)],
                        )
                    if strad:
                        # full row to host; symmetric strip source for all tiles
                        shf = scrp.tile([128, T], bf16, tag="shf", name=f"shf{s}_{rt}")
                        nc.scalar.copy(shf[:, 0:HC], pLo[:])
                        nc.scalar.copy(shf[:, HC:T], pHi[:])
                        nc.sync.dma_start(shipf[s][rt - t_lo], shf[:])
                        continue
                    if fake_t:
                        # v0 partial: min over the fake-certain zone, from PSUM
                        nc.vector.tensor_reduce(
                            stcs[s][:, 0, rt : rt + 1], pLo[:, 0:CF], axis=X, op=Alu.min
                        )
                    if fake_t or _ship_real(rt, s, t_hi):
                        shp = scrp.tile([128, 1024], bf16, tag="shp", name=f"shp{s}_{rt}")
                        nc.scalar.copy(shp[:, 0:WS], pHi[:, CRH:HC])
                        nc.sync.dma_start(shipd[s][rt], shp[:, 0:WS])
                    else:
                        # v2 partial: min over [CR:T) straight from PSUM
                        nc.vector.tensor_reduce(
                            stcs[s][:, 1, rt : rt + 1], pHi[:, CRH:HC], axis=X, op=Alu.min
                        )

            for s in range(BPC):
                nc.sync.dma_start(stats_c[s], stcs[s][:])

    nc.compile()
    return nc


def _prep(embeddings, label):
    """Host preprocessing: permutations, zone bounds, bf16 packed layout."""
    perms = np.empty((B, T), dtype=np.int64)
    nfs = np.empty(B, dtype=np.int64)
    for b in range(B):
        lb = label[b]
        perms[b] = np.argsort(lb, kind="stable")
        nfs[b] = int((lb == 0).sum())
    valid = (nfs > 0) & (nfs < T)
    if not valid.any():
        return None

    CF = int(nfs[valid].min())
    CR = int(nfs[valid].max())
    # the device reduces [0:CF) from the lo half and [CR:T) from the hi half,
    # so clamp the strip to bracket the 1024 boundary
    CF = max(2, min(CF, HC))
    CR = min(T - 2, max(CR, HC))
    t_lo = CF // 128
    t_hi = (CR + 127) // 128

    w = np.sqrt(np.sum(embeddings * embeddings, axis=-1, keepdims=True))
    n = embeddings / np.maximum(w, 1e-8)

    in_maps = []
    for c in range(NCORES):
        embt = np.empty((BPC, 128, T), dtype=ml_dtypes.bfloat16)
        for s in range(BPC):
            b = c * BPC + s
            embt[s] = n[b][perms[b]].T.astype(ml_dtypes.bfloat16)
        in_maps.append({"embt": embt})
    return perms, nfs, valid, CF, CR, t_lo, t_hi, in_maps


def kernel(embeddings, label):
    embeddings = np.ascontiguousarray(np.asarray(embeddings, dtype=np.float32))
    label = np.asarray(label)
    assert embeddings.shape == (B, T, D) and label.shape == (B, T)

    prep = _prep(embeddings, label)
    if prep is None:
        return np.float32(0.0)
    perms, nfs, valid, CF, CR, t_lo, t_hi, in_maps = prep

    nc = _build(CF, CR, t_lo, t_hi)

    from concourse.bass_utils import run_bass_kernel_spmd

    trace = bool(os.environ.get("CRL_TRACE"))
    if trace:
        _install_ntff_shim()
    res = run_bass_kernel_spmd(
        nc, in_maps, core_ids=list(range(NCORES)), trace=trace
    )
    if trace and res.exec_time_ns is not None:
        print(f"HW exec time: {res.exec_time_ns} ns")
        if res.instructions_and_trace:
            print("trace:", res.instructions_and_trace[1])

    # host tail: combine device stats, shipped [CR:T) slices, straddle rows
    base = t_lo * 128
    total = 0.0
    for c in range(NCORES):
        out = res.results[c]
        for s in range(BPC):
            b = c * BPC + s
            if not valid[b]:
                continue
            nf = int(nfs[b])
            stc = out["stats_c"][s].astype(np.float64)  # [128, 2, NT]
            Sstrip = out["shipf"][s].astype(np.float64).reshape(-1, T)
            shp = out["shipd"][s].astype(np.float64)    # [NT, 128, WS] cols CR:T

            minfake = np.full(T, np.inf)
            maxreal = np.full(T, -np.inf)
            minreal = np.full(T, np.inf)
            maxfake = np.full(T, -np.inf)

            # symmetric one-shot vectors over straddle rows (fake rows
            # [base:nf) for v0/v3 strip parts, real rows [nf:..) for v1/v2)
            FS = Sstrip[0 : nf - base]
            RS = Sstrip[nf - base :]
            M0 = FS.min(0) if len(FS) else np.full(T, np.inf)
            M1 = FS.max(0) if len(FS) else np.full(T, -np.inf)
            M2 = RS.max(0) if len(RS) else np.full(T, -np.inf)
            M3 = RS.min(0) if len(RS) else np.full(T, np.inf)
            # T0[j-CR]: max over pure-fake rows of col j (v3 source, j >= CR)
            if t_lo > 0:
                T0 = shp[0:t_lo].reshape(-1, T - CR).max(0)
            else:
                T0 = np.full(T - CR, -np.inf)

            for rt in range(NT128):
                rows = slice(rt * 128, (rt + 1) * 128)
                if t_lo <= rt < t_hi:  # straddle rows: direct from full rows
                    raw = Sstrip[(rt - t_lo) * 128 : (rt - t_lo + 1) * 128]
                    minfake[rows] = raw[:, 0:nf].min(-1)
                    maxfake[rows] = raw[:, 0:nf].max(-1)
                    minreal[rows] = raw[:, nf:T].min(-1)
                    maxreal[rows] = raw[:, nf:T].max(-1)
                    continue
                if rt < t_lo:  # pure fake rows: v0, v1
                    minfake[rows] = np.minimum(stc[:, 0, rt], M0[rows])
                    maxreal[rows] = np.maximum(shp[rt].max(-1), M2[rows])
                else:  # pure real rows: v2, v3
                    if _ship_real(rt, s, t_hi):
                        v2p = shp[rt].min(-1)
                    else:
                        v2p = stc[:, 1, rt]
                    minreal[rows] = np.minimum(v2p, M3[rows])
                    maxfake[rows] = np.maximum(
                        T0[rows.start - CR : rows.stop - CR], M1[rows]
                    )
            f2f = np.maximum(TH_SIM_MIN - minfake[:nf], 0.0).mean()
            r2r = np.maximum(TH_SIM_MIN - minreal[nf:], 0.0).mean()
            f2r = np.maximum(maxreal[:nf] - TH_DIFF_MAX, 0.0).mean()
            r2f = np.maximum(maxfake[nf:] - TH_DIFF_MAX, 0.0).mean()
            total += f2f + r2r + f2r + r2f
    return np.float32(total / B)


def _install_ntff_shim():
    """antenv.axon_hooks is missing on this image; inject it so trace=True works."""
    import types

    import antenv

    if hasattr(antenv, "axon_hooks"):
        return
    from trn_agent_boot.trn_boot import _ntff_profile_via_ctypes

    mod = types.ModuleType("antenv.axon_hooks")
    mod._hook = _ntff_profile_via_ctypes("/opt/axon/libaxon_pjrt.so")
    mod.get_axon_ntff_profile_hook = lambda: mod._hook
    mod.set_axon_ntff_profile_hook = lambda h: setattr(mod, "_hook", h)
    sys.modules["antenv.axon_hooks"] = mod
    antenv.axon_hooks = mod


# revision 31
# speedup vs baseline: 1.1428x; 1.1133x over previous
"""Trainium2 Bass kernel for nn_CRLoss (masked cosine-similarity contrastive loss).

Strategy (data-parallel over batch, 2 batches per core on 8 cores):
  Host: normalize rows in fp32, permute each batch's rows so label==0 ("fake")
  rows come first, cast to bf16, ship as [128, T] per batch.
  Device (per batch, 16 row-tiles of 128 rows):
    - fake row-tiles (rt < t_lo): 4 matmuls -> S[rows, 0:2048) in two
      [128,1024] PSUM halves; DVE tensor_reduce min over the fake-certain zone
      [0:CF) straight from PSUM (v0 partial); ACT casts the [CR:T) slice to
      bf16 and DMA ships it (it feeds the fake rows' own max-over-real stat
      AND, transposed on the host, the real rows' max-over-fake stat T0).
    - real row-tiles (rt >= t_hi): only 2 matmuls -> S[rows, 1024:2048).
      Most reduce min over [CR:T) on DVE straight from PSUM (v2 partial) and
      ship NOTHING; a few ship the [CR:T) slice instead to balance DVE vs ACT.
      The fake-zone max for real rows comes entirely from symmetry
      (S[j,c] = S[c,j]) via the fake tiles' shipped slices + straddle rows.
    - straddle row-tiles (mixed labels; they own ALL mixed-strip columns
      [CF:CR) as rows): 4 matmuls, full [0:T) row cast + DMA. These rows
      are the symmetric source for every tile's strip-column contributions.
  PSUM has exactly two consumers on trn2 (DVE one port, ACT; GPSIMD/DMA cannot
  read it), so the drain is split DVE/ACT and bulk data leaves as bf16.
  Host: min/max the shipped slices/rows in numpy (symmetric gathers), combine
  with device stats, then the reference's relu/mean/sum tail over B.
  bf16 shipping contributes ~2e-4 rel err (gate is 2e-2).
"""
import os
import sys

sys.path.insert(0, "/opt/trn_rl_repo")

import numpy as np
import ml_dtypes

B, T, D = 16, 2048, 128
NCORES = 8
BPC = B // NCORES  # batches per core
TH_SIM_MIN = 0.9
TH_DIFF_MAX = 0.1
NT128 = T // 128
HC = T // 2  # 1024: boundary between the lo and hi PSUM halves


def _ship_real(rt, s, t_hi):
    """Real tiles that ship their [CR:T) slice instead of reducing on DVE."""
    return ((rt - t_hi) * BPC + s) % 4 == 3


def _build(CF, CR, t_lo, t_hi):
    import concourse.bacc as bacc
    import concourse.mybir as mybir
    import concourse.tile as tile

    f32 = mybir.dt.float32
    bf16 = mybir.dt.bfloat16
    Alu = mybir.AluOpType
    X = mybir.AxisListType.X
    NSTR = t_hi - t_lo   # straddle tiles per slot
    ZF = t_lo * 128      # device fake zone [0:ZF); [ZF:nf) comes via shipf
    ZR = t_hi * 128      # ship/reduce start; [nf:ZR) comes via shipf
    CRH = ZR - HC        # [ZR:T) start within the hi half
    WS = T - ZR          # shipped slice width

    nc = bacc.Bacc("TRN2", target_bir_lowering=False)
    embt = nc.dram_tensor("embt", [BPC, 128, T], bf16, kind="ExternalInput")
    stats_c = nc.dram_tensor("stats_c", [BPC, 128, 2, NT128], f32, kind="ExternalOutput")
    shipd = nc.dram_tensor("shipd", [BPC, NT128, 128, WS], bf16, kind="ExternalOutput")
    shipf = nc.dram_tensor("shipf", [BPC, NSTR, 128, T], bf16, kind="ExternalOutput")

    with tile.TileContext(nc) as tc:
        with (
            tc.tile_pool(name="cst", bufs=1) as cst,
            tc.tile_pool(name="scr", bufs=3) as scrp,
            tc.tile_pool(name="stp", bufs=2) as stp,
            tc.tile_pool(name="ps", bufs=4, space="PSUM") as ps,
        ):
            nts = []
            for s in range(BPC):
                nt = cst.tile([128, T], bf16, tag=f"nt{s}", name=f"nt{s}")
                nts.append(nt)
            # chunk order: both slots' first chunks first, so the leading
            # matmuls of either slot unblock as early as possible (all on the
            # sync queue — gpsimd-issued DMAs measured slower end to end)
            for lo, hi in ((0, 512), (512, HC), (HC, T)):
                for s in range(BPC):
                    nc.sync.dma_start(nts[s][:, lo:hi], embt[s][:, lo:hi])

            stcs = []
            for s in range(BPC):
                stc = stp.tile([128, 2, NT128], f32, tag="stc", name=f"stc{s}")
                nc.gpsimd.memset(stc[:], 0.0)
                stcs.append(stc)

            # warmup matmuls on local garbage while the input DMAs are in
            # flight: the PE pstate needs ~3us of continuous busy to reach
            # 2.4GHz, so ramp it during the otherwise dead head
            warm = cst.tile([128, 512], bf16, tag="warm", name="warm")
            nc.gpsimd.memset(warm[:], 0.5)
            for i in range(8):
                pw = ps.tile([128, 512], f32, tag="ph", name=f"pwarm{i}")
                nc.tensor.matmul(pw[:], warm[:, 0:128], warm[:])

            # Tile order: alternate fake (4 matmuls) and real (2 matmuls)
            # tiles so PE production and consumer work stay balanced per pair;
            # straddle tiles (heavy ACT + big DMA) are spaced out.
            fakes = list(range(0, t_lo))
            reals = list(range(t_hi, NT128))
            strads = list(range(t_lo, t_hi))
            # lead with two fake tiles: they only need the first input chunks
            # (a real tile's hi half would stall on the last chunk to arrive)
            head, fk = fakes[:2], fakes[2:]
            order = list(head)
            for i in range(max(len(fk), len(reals))):
                if i < len(fk):
                    order.append(fk[i])
                if i < len(reals):
                    order.append(reals[i])
            for k, st in enumerate(strads):
                order.insert(
                    (k + 1) * (len(order) + len(strads)) // (len(strads) + 1), st
                )
            # end on reduce-only tiles (no cast, no ship DMA) so the post-PE
            # drain is short: pick real tiles that stay on DVE for both slots
            tail = [rt for rt in reals if all(
                not _ship_real(rt, s, t_hi) for s in range(BPC))][-2:]
            order = [rt for rt in order if rt not in tail] + tail

            for rt in order:
                for s in range(BPC):
                    nt = nts[s]
                    fake_t = rt < t_lo
                    strad = t_lo <= rt < t_hi
                    lhsT = nt[:, rt * 128 : (rt + 1) * 128]
                    pLo = None
                    if fake_t or strad:
                        pLo = ps.tile([128, HC], f32, tag="ph", name=f"pLo{s}_{rt}")
                        for j in range(2):
                            nc.tensor.matmul(
                                pLo[:, 512 * j : 512 * (j + 1)],
                                lhsT,
                                nt[:, 512 * j : 512 * (j + 1)],
                            )
                    pHi = ps.tile([128, HC], f32, tag="ph", name=f"pHi{s}_{rt}")
                    for j in range(2):
                        nc.tensor.matmul(
                            pHi[:, 512 * j : 512 * (j + 1)],
                            lhsT,
                            nt[:, HC + 512 * j : HC + 512 * (j + 1)],
                        )
                    if strad:
                        # full row to host; symmetric strip source for all tiles
                        shf = scrp.tile([128, T], bf16, tag="shf", name=f"shf{s}_{rt}")
                        nc.scalar.copy(shf[:, 0:HC], pLo[:])
                        nc.scalar.copy(shf[:, HC:T], pHi[:])
                        nc.sync.dma_start(shipf[s][rt - t_lo], shf[:])
                        continue
                    if fake_t:
                        # v0 partial: min over the fake-certain zone, from PSUM
                        nc.vector.tensor_reduce(
                            stcs[s][:, 0, rt : rt + 1], pLo[:, 0:ZF], axis=X, op=Alu.min
                        )
                    if fake_t or _ship_real(rt, s, t_hi):
                        shp = scrp.tile([128, 1024], bf16, tag="shp", name=f"shp{s}_{rt}")
                        nc.scalar.copy(shp[:, 0:WS], pHi[:, CRH:HC])
                        nc.sync.dma_start(shipd[s][rt], shp[:, 0:WS])
                    else:
                        # v2 partial: min over [CR:T) straight from PSUM
                        nc.vector.tensor_reduce(
                            stcs[s][:, 1, rt : rt + 1], pHi[:, CRH:HC], axis=X, op=Alu.min
                        )

            for s in range(BPC):
                nc.sync.dma_start(stats_c[s], stcs[s][:])

    nc.compile()
    return nc


def _prep(embeddings, label):
    """Host preprocessing: permutations, zone bounds, bf16 packed layout."""
    perms = np.empty((B, T), dtype=np.int64)
    nfs = np.empty(B, dtype=np.int64)
    for b in range(B):
        lb = label[b]
        perms[b] = np.argsort(lb, kind="stable")
        nfs[b] = int((lb == 0).sum())
    valid = (nfs > 0) & (nfs < T)
    if not valid.any():
        return None

    CF = int(nfs[valid].min())
    CR = int(nfs[valid].max())
    # the device reduces [0:CF) from the lo half and [CR:T) from the hi half,
    # so clamp the strip to bracket the 1024 boundary
    CF = max(2, min(CF, HC))
    CR = min(T - 2, max(CR, HC))
    t_lo = CF // 128
    t_hi = (CR + 127) // 128

    w = np.sqrt(np.sum(embeddings * embeddings, axis=-1, keepdims=True))
    n = embeddings / np.maximum(w, 1e-8)

    in_maps = []
    for c in range(NCORES):
        embt = np.empty((BPC, 128, T), dtype=ml_dtypes.bfloat16)
        for s in range(BPC):
            b = c * BPC + s
            embt[s] = n[b][perms[b]].T.astype(ml_dtypes.bfloat16)
        in_maps.append({"embt": embt})
    return perms, nfs, valid, CF, CR, t_lo, t_hi, in_maps


def kernel(embeddings, label):
    embeddings = np.ascontiguousarray(np.asarray(embeddings, dtype=np.float32))
    label = np.asarray(label)
    assert embeddings.shape == (B, T, D) and label.shape == (B, T)

    prep = _prep(embeddings, label)
    if prep is None:
        return np.float32(0.0)
    perms, nfs, valid, CF, CR, t_lo, t_hi, in_maps = prep

    nc = _build(CF, CR, t_lo, t_hi)

    from concourse.bass_utils import run_bass_kernel_spmd

    trace = bool(os.environ.get("CRL_TRACE"))
    if trace:
        _install_ntff_shim()
    res = run_bass_kernel_spmd(
        nc, in_maps, core_ids=list(range(NCORES)), trace=trace
    )
    if trace and res.exec_time_ns is not None:
        print(f"HW exec time: {res.exec_time_ns} ns")
        if res.instructions_and_trace:
            print("trace:", res.instructions_and_trace[1])

    # host tail: combine device stats, shipped [CR:T) slices, straddle rows
    base = t_lo * 128
    total = 0.0
    for c in range(NCORES):
        out = res.results[c]
        for s in range(BPC):
            b = c * BPC + s
            if not valid[b]:
                continue
            nf = int(nfs[b])
            stc = out["stats_c"][s].astype(np.float64)  # [128, 2, NT]
            Sstrip = out["shipf"][s].astype(np.float64).reshape(-1, T)
            shp = out["shipd"][s].astype(np.float64)    # [NT, 128, WS] cols ZR:T
            ZR = t_hi * 128

            minfake = np.full(T, np.inf)
            maxreal = np.full(T, -np.inf)
            minreal = np.full(T, np.inf)
            maxfake = np.full(T, -np.inf)

            # symmetric one-shot vectors over straddle rows (fake rows
            # [base:nf) for v0/v3 strip parts, real rows [nf:..) for v1/v2)
            FS = Sstrip[0 : nf - base]
            RS = Sstrip[nf - base :]
            M0 = FS.min(0) if len(FS) else np.full(T, np.inf)
            M1 = FS.max(0) if len(FS) else np.full(T, -np.inf)
            M2 = RS.max(0) if len(RS) else np.full(T, -np.inf)
            M3 = RS.min(0) if len(RS) else np.full(T, np.inf)
            # T0[j-ZR]: max over pure-fake rows of col j (v3 source, j >= ZR)
            if t_lo > 0:
                T0 = shp[0:t_lo].reshape(-1, T - ZR).max(0)
            else:
                T0 = np.full(T - ZR, -np.inf)

            for rt in range(NT128):
                rows = slice(rt * 128, (rt + 1) * 128)
                if t_lo <= rt < t_hi:  # straddle rows: direct from full rows
                    raw = Sstrip[(rt - t_lo) * 128 : (rt - t_lo + 1) * 128]
                    minfake[rows] = raw[:, 0:nf].min(-1)
                    maxfake[rows] = raw[:, 0:nf].max(-1)
                    minreal[rows] = raw[:, nf:T].min(-1)
                    maxreal[rows] = raw[:, nf:T].max(-1)
                    continue
                if rt < t_lo:  # pure fake rows: v0, v1
                    minfake[rows] = np.minimum(stc[:, 0, rt], M0[rows])
                    maxreal[rows] = np.maximum(shp[rt].max(-1), M2[rows])
                else:  # pure real rows: v2, v3
                    if _ship_real(rt, s, t_hi):
                        v2p = shp[rt].min(-1)
                    else:
                        v2p = stc[:, 1, rt]
                    minreal[rows] = np.minimum(v2p, M3[rows])
                    maxfake[rows] = np.maximum(
                        T0[rows.start - ZR : rows.stop - ZR], M1[rows]
                    )
            f2f = np.maximum(TH_SIM_MIN - minfake[:nf], 0.0).mean()
            r2r = np.maximum(TH_SIM_MIN - minreal[nf:], 0.0).mean()
            f2r = np.maximum(maxreal[:nf] - TH_DIFF_MAX, 0.0).mean()
            r2f = np.maximum(maxfake[nf:] - TH_DIFF_MAX, 0.0).mean()
            total += f2f + r2r + f2r + r2f
    return np.float32(total / B)


def _install_ntff_shim():
    """antenv.axon_hooks is missing on this image; inject it so trace=True works."""
    import types

    import antenv

    if hasattr(antenv, "axon_hooks"):
        return
    from trn_agent_boot.trn_boot import _ntff_profile_via_ctypes

    mod = types.ModuleType("antenv.axon_hooks")
    mod._hook = _ntff_profile_via_ctypes("/opt/axon/libaxon_pjrt.so")
    mod.get_axon_ntff_profile_hook = lambda: mod._hook
    mod.set_axon_ntff_profile_hook = lambda h: setattr(mod, "_hook", h)
    sys.modules["antenv.axon_hooks"] = mod
    antenv.axon_hooks = mod
